# revision 60
# baseline (speedup 1.0000x reference)
"""Multi-head attention (B=2, N=2048, C=512, H=8) on 8 trn2 NeuronCores.

Sharding: tensor-parallel over heads x data-parallel over batch.
Core i handles batch b = i//4 and heads {2*(i%4), 2*(i%4)+1} (a contiguous
128-column slice of Wq/Wk/Wv and 128-row slice of Wo). Each core computes
its heads' full attention and a partial output projection; the host sums
the 4 partials per batch (adding bo once) and stacks batches.

Design (baseline 191us -> 137us -> this version):
  - Host pre-transposes q/kv/pos to channel-major and pre-casts all matmul
    operands to fp16 (no on-device input transposes, half the DMA bytes).
  - fp16 matmul operands everywhere, fp32 PSUM accumulation.
  - exp split across two engines per (kc, head) by parity: ACT table EXP
    vs a DVE Schraudolph exp (tensor_scalar -> int16 bit pattern IS fp16
    exp; softmax renormalization mostly cancels the 4% element error).
  - HAM-warmth is the key constraint: the PE clock halves (2.4->1.2GHz)
    whenever the PE micro-idles through a 3.4us activity window. So ACT
    and DVE carry NOTHING but exp + PSUM evacuations during the attention
    stream: every DMA trigger lives on the SP(sync) or Pool(gpsimd)
    queues, the softmax reciprocal is computed at [128,16] (DRAM-bounce
    reshape) instead of [64,2048], and evacuations alternate engines.
  - Phases 1+2 fused: attention k-chunks emit as soon as their projection
    slabs exist; slab0 is emitted as two 256-token halves so the PE
    starts right behind the first DMA chunks.
  - S quads issue qq-outer [h0q0,h1q0,h0q1,h1q1]: the two heads' S
    matmuls run CONCURRENTLY on disjoint PE row groups (tile_position
    auto-derived from khT base partitions 0/64) -> ~250ns per pair.
  - One shared 4-bank PSUM work pool (2 lineages x bufs=2) carries
    projection groups, vp transposes, S tiles AND phase-3 Y tiles;
    + 2x[65,1024] PV accumulators = exactly 8 banks.
  - PV lhsT is [128,65] ([d|ones]): the ones column yields softmax
    denominators in the PV accumulator for free.
  - Normalized O is written STACKED: norm writes head h into partitions
    64h..64h+63 of O2[128, N] (DVE ops may shift output partition base),
    so the output projection is 16 contract-128 matmuls instead of 32
    contract-64 ones.
  - PV issue lags S/exp by PVLAG k-chunks; normalize DVE work is
    deferred past the next q-half's first exps.
  - fp16 output partials; output DMA alternates sync/gpsimd queues.
"""
import numpy as np

B, N, C, H, D = 2, 2048, 512, 8, 64
SCALE = float(C) ** -0.5
NCORES = 8
P = 128
CC = C // P            # 4 channel chunks of 128
NT16 = N // P          # 16 token tiles of 128
NT4 = N // 512         # 4 token slabs of 512
PVLAG = 4              # PV issue lag in k-chunks

# Schraudolph fp16 exp constants (attention scale folded in):
#   i16 = rni(s * SCH_A + SCH_B); bitcast f16 ~= exp(s * SCALE)
SCH_A = SCALE * (2.0 ** 10) / float(np.log(2.0))
SCH_B = 15.0 * 1024.0 - 486411.0 / 8192.0

_cached_nc = None


def _build():
    from contextlib import ExitStack

    import concourse.mybir as mybir
    import concourse.tile as tile
    from concourse import bacc
    from concourse.alu_op_type import AluOpType
    from concourse.tile_rust import add_dep_helper

    f32 = mybir.dt.float32
    f16 = mybir.dt.float16
    i16 = mybir.dt.int16
    AF = mybir.ActivationFunctionType

    nc = bacc.Bacc("TRN2", target_bir_lowering=False, debug=False)

    qT = nc.dram_tensor("qT", [P, CC, N], f16, kind="ExternalInput")
    kvT = nc.dram_tensor("kvT", [P, CC, N], f16, kind="ExternalInput")
    posq = nc.dram_tensor("posq", [64, N], f16, kind="ExternalInput")
    posk = nc.dram_tensor("posk", [64, N], f16, kind="ExternalInput")
    wq = nc.dram_tensor("wq", [P, CC, P], f16, kind="ExternalInput")
    wk = nc.dram_tensor("wk", [P, CC, P], f16, kind="ExternalInput")
    wv = nc.dram_tensor("wv", [P, CC, P], f16, kind="ExternalInput")
    wo = nc.dram_tensor("wo", [P, C], f16, kind="ExternalInput")
    eye = nc.dram_tensor("eye", [P, P], f16, kind="ExternalInput")
    eyepos = nc.dram_tensor("eyepos", [64, P], f16, kind="ExternalInput")
    y = nc.dram_tensor("y", [N, C], f16, kind="ExternalOutput")

    with tile.TileContext(nc) as tc, ExitStack() as ctx:
        persist = ctx.enter_context(tc.tile_pool(name="persist", bufs=1))

        # ---- persistent SBUF tiles ----
        qT_sb = persist.tile([P, CC, N], f16, tag="qT_sb")
        kvT_sb = persist.tile([P, CC, N], f16, tag="kvT_sb")
        w_sb = {name: persist.tile([P, CC, P], f16, tag=f"{name}_sb",
                                   name=f"{name}_sb")
                for name in ("wq", "wk", "wv")}
        pos_sb = {name: persist.tile([P, N], f16, tag=f"pos_{name}",
                                     name=f"pos_{name}")
                  for name in ("q", "k")}
        eye_sb = persist.tile([P, P], f16, tag="eye_sb")
        ep_sb = persist.tile([64, P], f16, tag="ep_sb")
        # ones row AT partition 64 (must match den_r's partition for the
        # K=1 broadcast matmul's tile_position derivation)
        ones_t = persist.tile([P, 64], f16, tag="ones_t")
        wo2 = persist.tile([P, C], f16, tag="wo2")
        qhT = persist.tile([P, N], f16, tag="qhT")
        khT = persist.tile([P, N], f16, tag="khT")
        # normalized output, stacked heads (h on partitions 64h..64h+63),
        # split per q-half so phase-3 y units of half 0 carry no dependency
        # on half 1's normalize
        O2 = [persist.tile([P, N // 2], f16, tag=f"O2{i}", name=f"O2{i}")
              for i in (0, 1)]
        vpT = persist.tile([P, N], f16, tag="vpT")
        # vp layout per (kc, head): [d0..63 | ones] -> [128, 65] PV lhsT
        vp4 = persist.tile([P, NT16, 2, 65], f16, tag="vp4")

        # ---- input DMAs: SP queue carries the q path, Pool queue the kv
        # path; the ACT queue carries NO DMA so exp is never trigger-stalled.
        # Order = priority: wq + first q chunks first, then alternate so the
        # fused proj stream unblocks slab by slab.
        nc.sync.dma_start(w_sb["wq"][:], wq[:])
        nc.sync.dma_start(qT_sb[:, :, 0:256], qT[:, :, 0:256])
        nc.sync.dma_start(qT_sb[:, :, 256:512], qT[:, :, 256:512])
        nc.sync.dma_start(pos_sb["q"][0:64, 0:1024], posq[:, 0:1024])
        nc.sync.dma_start(qT_sb[:, :, 512:768], qT[:, :, 512:768])
        nc.sync.dma_start(qT_sb[:, :, 768:1024], qT[:, :, 768:1024])
        nc.sync.dma_start(qT_sb[:, :, 1024:1536], qT[:, :, 1024:1536])
        nc.sync.dma_start(pos_sb["q"][0:64, 1024:2048], posq[:, 1024:2048])
        nc.sync.dma_start(qT_sb[:, :, 1536:2048], qT[:, :, 1536:2048])

        nc.gpsimd.dma_start(ep_sb[:], eyepos[:])
        nc.gpsimd.dma_start(w_sb["wk"][:], wk[:])
        nc.gpsimd.dma_start(kvT_sb[:, :, 0:256], kvT[:, :, 0:256])
        nc.gpsimd.dma_start(kvT_sb[:, :, 256:512], kvT[:, :, 256:512])
        nc.gpsimd.dma_start(w_sb["wv"][:], wv[:])
        nc.gpsimd.dma_start(pos_sb["k"][0:64, 0:1024], posk[:, 0:1024])
        nc.gpsimd.dma_start(eye_sb[:], eye[:])
        nc.gpsimd.dma_start(kvT_sb[:, :, 512:1024], kvT[:, :, 512:1024])
        nc.gpsimd.dma_start(kvT_sb[:, :, 1024:1536], kvT[:, :, 1024:1536])
        nc.gpsimd.dma_start(pos_sb["k"][0:64, 1024:2048], posk[:, 1024:2048])
        nc.gpsimd.dma_start(kvT_sb[:, :, 1536:2048], kvT[:, :, 1536:2048])
        nc.gpsimd.dma_start(wo2[:], wo[:])
        # duplicate pos to partitions 64-127 (head-pair broadcast, used by
        # the DVE-add projection path of slabs 2-3)
        nc.gpsimd.dma_start(pos_sb["q"][64:128, :], pos_sb["q"][0:64, :])
        nc.gpsimd.dma_start(pos_sb["k"][64:128, :], pos_sb["k"][0:64, :])
        nc.gpsimd.memset(vp4[:, :, :, 64:65], 1.0)
        nc.gpsimd.memset(ones_t[64:65, :], 1.0)

        # PE order pinned with order-only deps (the PE queue is in-order).
        pe_prev = [None]

        def chain(mm):
            if pe_prev[0] is not None:
                add_dep_helper(mm.ins, pe_prev[0].ins, sync=False,
                               reason="pin PE order")
            pe_prev[0] = mm

        # ---- pools: 4-bank shared work pool + 4-bank PV accumulators ----
        work_ps = ctx.enter_context(
            tc.tile_pool(name="work_ps", bufs=2, space="PSUM"))
        ot_ps = ctx.enter_context(
            tc.tile_pool(name="ot_ps", bufs=1, space="PSUM"))
        expp = ctx.enter_context(tc.tile_pool(name="expp", bufs=12))
        den_pool = ctx.enter_context(tc.tile_pool(name="den", bufs=1))
        den_dram = ctx.enter_context(
            tc.tile_pool(name="dend", bufs=2, space="DRAM"))
        yout = ctx.enter_context(tc.tile_pool(name="yout", bufs=4))

        wk_ctr = [0]

        def work_tile(shape, dt, name):
            tag = f"st{wk_ctr[0] % 2}"
            wk_ctr[0] += 1
            return work_ps.tile(shape, dt, tag=tag, name=name)

        # ---- phase-1 emission units (interleavable into the kc stream) --
        def emit_proj(lo, hi, wname):
            sl = slice(lo, hi)
            srcT = qT_sb if wname == "wq" else kvT_sb
            dstT, posn = ((qhT, "q") if wname == "wq" else
                          (khT, "k") if wname == "wk" else (None, None))
            # slabs 0-1 q/k: fold the pos-add into the matmul group via an
            # [I64|I64] lhsT so the evacuation is an ACT copy and the DVE
            # queue stays clear for the first Schraudolph exps
            pe_pos = dstT is not None and hi <= 1024
            pp = work_tile([P, hi - lo], f32, f"pp_{wname}{lo}")
            for cc in range(CC):
                chain(nc.tensor.matmul(
                    pp[:], w_sb[wname][:, cc], srcT[:, cc, sl],
                    start=(cc == 0), stop=(cc == CC - 1 and not pe_pos)))
            if pe_pos:
                chain(nc.tensor.matmul(
                    pp[:], ep_sb[:], pos_sb[posn][0:64, sl],
                    start=False, stop=True))
                nc.scalar.copy(dstT[:, sl], pp[:])
            elif dstT is not None:
                nc.vector.tensor_add(
                    out=dstT[:, sl], in0=pp[:], in1=pos_sb[posn][:, sl])
            else:
                nc.scalar.copy(vpT[:, sl], pp[:])

        def emit_tp(t):
            tp = work_tile([P, P], f16, f"tp{t}")
            chain(nc.tensor.matmul(
                tp[:], vpT[:, t * P:(t + 1) * P], eye_sb[:],
                is_transpose=True))
            src = tp[:].rearrange("p (h d) -> p h d", h=2)
            if t % 2 == 0:
                nc.vector.tensor_copy(vp4[:, t, :, 0:64], src)
            else:
                nc.scalar.copy(vp4[:, t, :, 0:64], src)

        def emit_qk(lo, hi):
            emit_proj(lo, hi, "wq")
            emit_proj(lo, hi, "wk")

        def emit_v(lo, hi):
            emit_proj(lo, hi, "wv")
            for t in range(lo // P, hi // P):
                emit_tp(t)

        # ---- phase-2 emission ----
        st2 = {"ot": None, "exq": None}

        def s_quad(qh2, kc):
            # qq-outer order [h0q0, h1q0, h0q1, h1q1]: head pairs execute
            # concurrently on disjoint PE row groups.
            exs = [expp.tile([P, 1024], f16, tag=f"ex{h}", name=f"ex{h}")
                   for h in (0, 1)]
            for qq in range(2):
                for h in (0, 1):
                    hsl = slice(64 * h, 64 * h + 64)
                    st = work_ps.tile([P, 512], f32, tag=f"st{h}",
                                      name=f"st{h}q{qq}")
                    chain(nc.tensor.matmul(
                        st[:],
                        khT[hsl, kc * P:(kc + 1) * P],
                        qhT[hsl, (qh2 * 2 + qq) * 512:
                                 (qh2 * 2 + qq + 1) * 512],
                        start=True, stop=True))
                    exq_sl = exs[h][:, qq * 512:(qq + 1) * 512]
                    if (kc + h) % 2 == 0:
                        nc.scalar.activation(exq_sl, st[:], AF.Exp,
                                             scale=SCALE)
                    else:
                        nc.vector.tensor_scalar(
                            exq_sl.bitcast(i16), st[:], SCH_A, SCH_B,
                            AluOpType.mult, AluOpType.add)
            return exs

        def pv_quad(kc, exs):
            OT = st2["ot"]
            for h in (0, 1):
                for qq in range(2):
                    chain(nc.tensor.matmul(
                        OT[h][:, qq * 512:(qq + 1) * 512],
                        vp4[:, kc, h, :],
                        exs[h][:, qq * 512:(qq + 1) * 512],
                        start=(kc == 0), stop=(kc == NT16 - 1)))

        def emit_kc(qh2, kc):
            if kc == 0:
                st2["ot"] = [ot_ps.tile([65, 1024], f32, tag=f"ot{h}",
                                        name=f"ot{h}") for h in (0, 1)]
                st2["exq"] = []
                st2["pv"] = 0
            st2["exq"].append(s_quad(qh2, kc))
            if qh2 == 0:
                # steady lag: PV k-chunks chase the exp stream
                if kc >= PVLAG:
                    pv_quad(kc - PVLAG, st2["exq"][kc - PVLAG])
                    st2["pv"] = kc - PVLAG + 1
            else:
                # qh2=1's first PV WAR-waits normalize(0)'s reads of the old
                # accumulators (~3.5us chain); give it a 4-kc runway and
                # catch up with two PV quads per kc
                if kc >= 4:
                    for _ in range(2):
                        if st2["pv"] <= min(2 * (kc - 4) + 1, kc - 2):
                            pv_quad(st2["pv"], st2["exq"][st2["pv"]])
                            st2["pv"] += 1

        def drain_pv():
            for kc in range(st2["pv"], NT16):
                pv_quad(kc, st2["exq"][kc])

        # normalize part A: den-row copies to f16 SBUF (split across ACT/DVE)
        def normalizeA(qh2):
            OT = st2["ot"]
            den_r = den_pool.tile([P, 2, 1024], f16, tag="den_r")
            nc.scalar.copy(den_r[64:65, 0, :], OT[0][64:65, :])
            nc.vector.tensor_copy(den_r[64:65, 1, :], OT[1][64:65, :])
            return OT, den_r

        # normalize part B, per (head, 512-col half): broadcast the den row
        # across 64 partitions with a K=1 PE matmul (ones column x den row;
        # ~250ns, no DRAM bounce), then one DVE reciprocal on the broadcast.
        # (reciprocal_approx_fast is silently wrong on single-partition APs,
        # hence recip-after-broadcast.)
        def norm_bc(state, h, half):
            sl = slice(half * 512, (half + 1) * 512)
            bc = work_tile([64, 512], f32, f"bc{h}{half}")
            chain(nc.tensor.matmul(
                bc[:], ones_t[64:65, :], state[1][64:65, h, sl],
                start=True, stop=True))
            rec = den_pool.tile([64, 512], f32, tag=f"rec{h}{half}")
            nc.vector.reciprocal_approx_fast(rec[:], bc[:])
            return rec

        def norm_mult(qh2, state, rec, h, half):
            # head h lands on partitions 64h..64h+63 of O2 (stacked layout);
            # emitted per 512-col half so downstream y units release early
            sl = slice(half * 512, (half + 1) * 512)
            nc.vector.tensor_mul(
                out=O2[qh2][64 * h:64 * h + 64, sl],
                in0=state[0][h][0:64, sl], in1=rec[:])

        # ---- fused emission schedule: only the q/k projections of slabs
        # 0-1 precede the first S quads; everything else (v projections,
        # transposes, slabs 2-3) is spread one unit per k-chunk so the exp
        # stream never pauses and the PE fills its exp-floor slack ----
        emit_qk(0, 256)
        emit_qk(256, 512)
        emit_qk(512, 768)
        emit_qk(768, 1024)
        emit_kc(0, 0)
        emit_v(0, 256)
        emit_kc(0, 1)
        emit_v(256, 512)
        emit_kc(0, 2)
        emit_v(512, 768)
        emit_kc(0, 3)
        emit_v(768, 1024)
        for i, wname in enumerate(("wq", "wk", "wv")):
            emit_kc(0, 4 + i)
            emit_proj(1024, 1536, wname)
        emit_kc(0, 7)
        emit_tp(8)
        emit_tp(9)
        emit_kc(0, 8)
        emit_tp(10)
        emit_tp(11)
        for i, wname in enumerate(("wq", "wk", "wv")):
            emit_kc(0, 9 + i)
            emit_proj(1536, 2048, wname)
        emit_kc(0, 12)
        emit_tp(12)
        emit_tp(13)
        emit_kc(0, 13)
        emit_tp(14)
        emit_tp(15)
        emit_kc(0, 14)
        emit_kc(0, 15)
        drain_pv()
        norm0 = normalizeA(0)
        emit_kc(1, 0)
        r00 = norm_bc(norm0, 0, 0)
        norm_mult(0, norm0, r00, 0, 0)
        emit_kc(1, 1)
        r01 = norm_bc(norm0, 0, 1)
        norm_mult(0, norm0, r01, 0, 1)
        emit_kc(1, 2)
        r10 = norm_bc(norm0, 1, 0)
        norm_mult(0, norm0, r10, 1, 0)
        emit_kc(1, 3)
        r11 = norm_bc(norm0, 1, 1)
        norm_mult(0, norm0, r11, 1, 1)
        for kc in range(4, NT16):
            emit_kc(1, kc)
        drain_pv()

        # ---- phase 3: output projection (partials, bias added on host) --
        # stacked O2 -> one contract-128 matmul per token tile; token-tile
        # pairs share one PSUM tile and one evacuation copy
        deferred_ydma = []

        def y_unit(tt, pool, defer_dma=False):
            ysb = yout.tile([P, 2, C], f16, tag="ysb")
            if pool is work_ps:
                yps = [work_tile([P, C], f32, f"yp{2 * tt + ti}")[:]
                       for ti in range(2)]
            else:
                yp2 = ot_ps.tile([P, 2, C], f32, tag=f"ot{tt % 2}",
                                 name=f"yp2_{tt}")
                yps = [yp2[:, 0, :], yp2[:, 1, :]]
            for ti in range(2):
                t = 2 * tt + ti
                tsl = slice((t % 8) * P, (t % 8 + 1) * P)
                chain(nc.tensor.matmul(
                    yps[ti], O2[t // 8][:, tsl], wo2[:], start=True,
                    stop=True))
            if pool is work_ps:
                nc.scalar.copy(ysb[:, 0, :], yps[0])
                nc.vector.tensor_copy(ysb[:, 1, :], yps[1])
            elif tt % 2 == 0:
                nc.scalar.copy(ysb[:], yp2[:])
            else:
                nc.vector.tensor_copy(ysb[:], yp2[:])
            # all y DMA on the Pool queue; the SP queue owns the rec chains.
            # y0-3's DMAs are deferred past the half-1 rec chain so their
            # 256KB transfers don't contend with the chain's tiny hops on
            # the shared DMA engines.
            ydst = y[2 * tt * P:(2 * tt + 2) * P, :].rearrange(
                "(a p) c -> p a c", a=2)
            if defer_dma:
                deferred_ydma.append((ydst, ysb))
            else:
                nc.gpsimd.dma_start(ydst, ysb[:])

        # half-1 normalize chain first (den copies sort ahead of y evacs on
        # the in-order ACT/DVE queues, so the chain starts the moment the
        # last PV lands); y units 0-3 depend only on half 0 and fill the PE
        # while the chain flies; y4/y5 release after the first (h0,h1) halves
        norm1 = normalizeA(1)
        y_unit(0, work_ps)
        s00 = norm_bc(norm1, 0, 0)
        norm_mult(1, norm1, s00, 0, 0)
        y_unit(1, work_ps)
        s10 = norm_bc(norm1, 1, 0)
        norm_mult(1, norm1, s10, 1, 0)
        y_unit(2, work_ps)
        s01 = norm_bc(norm1, 0, 1)
        norm_mult(1, norm1, s01, 0, 1)
        y_unit(3, work_ps)
        # y4/y5 go through work_ps: the ot_ps banks still hold the OT
        # accumulators until the half-1 mults read them
        y_unit(4, work_ps)
        s11 = norm_bc(norm1, 1, 1)
        norm_mult(1, norm1, s11, 1, 1)
        y_unit(5, work_ps)
        y_unit(6, ot_ps)
        y_unit(7, ot_ps)

    nc.finalize()
    return nc


def _chmajor(x):
    # [N, C] token-major f32 -> [P, CC, N] channel-major chunked f16
    return np.ascontiguousarray(
        x.T.reshape(CC, P, N).transpose(1, 0, 2)).astype(np.float16)


def _wchunk(w):
    # [C, P] -> [P, CC, P] lhsT chunks
    return np.ascontiguousarray(
        w.reshape(CC, P, P).transpose(1, 0, 2)).astype(np.float16)


def _in_maps(q, kv, pos_q, pos_k, Wq, Wk, Wv, Wo, bo):
    maps = []
    for i in range(NCORES):
        b, hp = i // 4, i % 4
        cs = P * hp
        maps.append({
            "qT": _chmajor(np.asarray(q[b], dtype=np.float32)),
            "kvT": _chmajor(np.asarray(kv[b], dtype=np.float32)),
            "posq": np.ascontiguousarray(pos_q[b].T).astype(np.float16),
            "posk": np.ascontiguousarray(pos_k[b].T).astype(np.float16),
            "wq": _wchunk(np.asarray(Wq[:, cs:cs + P], dtype=np.float32)),
            "wk": _wchunk(np.asarray(Wk[:, cs:cs + P], dtype=np.float32)),
            "wv": _wchunk(np.asarray(Wv[:, cs:cs + P], dtype=np.float32)),
            "wo": np.ascontiguousarray(Wo[cs:cs + P, :]).astype(np.float16),
            "eye": np.eye(P, dtype=np.float16),
            "eyepos": np.ascontiguousarray(np.concatenate(
                [np.eye(64), np.eye(64)], axis=1)).astype(np.float16),
        })
    return maps


def kernel(q, kv, pos_q, pos_k, Wq, Wk, Wv, Wo, bo):
    from concourse.bass_utils import run_bass_kernel_spmd

    global _cached_nc
    if _cached_nc is None:
        _cached_nc = _build()

    args = [np.asarray(a) for a in (q, kv, pos_q, pos_k, Wq, Wk, Wv, Wo, bo)]
    maps = _in_maps(*args)
    res = run_bass_kernel_spmd(_cached_nc, maps, list(range(NCORES)))
    outs = [res.results[i]["y"].astype(np.float32) for i in range(NCORES)]
    bo32 = np.asarray(args[8], dtype=np.float32)
    y0 = outs[0] + outs[1] + outs[2] + outs[3] + bo32
    y1 = outs[4] + outs[5] + outs[6] + outs[7] + bo32
    return np.stack([y0, y1]).astype(np.float32)
